# revision 1
# baseline (speedup 1.0000x reference)
"""Trainium2 Bass kernel for nn_AttentionHead (B=4, n_ctx=4096, d_model=1024,
d_hidden=64, causal, scale=1/sqrt(d_model)).

Sharding: 8 cores = 4 batches x 2 balanced causal shards. Core (b, s) handles
the 2048 query rows in 64-row chunks with chunk%2 == s. Keys/x-columns are
permuted per core (my-parity chunks first within each 512-key ntile) so that
every core runs the IDENTICAL SPMD program:

  - slot j (0..3) = 512 queries = my chunks of ntiles 2j, 2j+1
  - slot j attends k-tiles t = 0..8(j+1)-1 (128 permuted keys each)
  - k-tiles t < 8j are fully open; t = 8j + r (r in 0..7) get an additive
    causal mask that depends only on (r, s) -> 8 mask tiles per core, sent
    as data.

Per-core pipeline (all matmuls in float32r: 1 cycle/row at N>=256):
  A: KT/VT = [Wk;Wov] @ xT (weights stationary, PSUM-accumulated over 8
     d_model chunks, biases folded in as K=1 rank-1 matmuls against a ones
     row); Q likewise from each ntile's first 256 columns (= my 4 chunks).
     V transposed to natural [k,64] layout via PE transpose, with an
     appended ones column so attn@[V,1] also yields the softmax denominator.
  B: S^T[k,q] = KT_tile^T @ QT_slot -> PSUM (two k-tiles paired per 2-bank
     PSUM tile); additive mask via identity matmul for diagonal tiles;
     exp((S+M)/32) on ACT over the pair -> SBUF (no row-max subtraction
     needed: |scores/32| <~ 1.5).
  C: O65 += V65_tile^T @ E (PSUM accumulate over k-tiles); row 64 = denom.
  D: y_tile = (O65_slice^T @ [Wo^T; bo]) * (1/den) -- den row makes the
     matmul add den*bo, so the per-partition reciprocal multiply yields
     O@Wo^T/den + bo exactly. Reciprocals come from 16 PE transposes of the
     denominator row into [128,16] + one DVE reciprocal.

DMA instruction count is kept low (each DMA holds the shared HWDGE
descriptor generator ~625ns): x arrives bf16 in an ntile-major host layout
(two [128,4x512] loads per 512-key ntile, so each ntile's projections and
the attention slots that depend on them cascade right behind the DMA
stream), y leaves as 16 [128,1024] stores, constants are consolidated
single loads ordered by first use. Emission is interleaved (A ntiles,
then each slot's B/C with the previous slot's D inside) and C trails one
pair behind B/exp so no engine's in-order stream blocks on another.
"""

import math

import numpy as np

D = 1024
H = 64
N = 4096
B = 4
CH = 64  # query chunk size (rows)
NT = 8  # ntiles of 512 keys
NEG = -1e10
# per masked-tile r: length of the fully-dead leading q-column prefix, min
# over both core parities, clamped to 256 (fp32r full-rate floor)
MASK_OFFS = [0, 128, 0, 128, 256, 256, 256, 256]
# per masked-tile r: end of the nonzero mask band (max over parities); the
# mask-add matmul only needs to cover [MASK_OFFS[r], MASK_ENDS[r])
MASK_ENDS = [127, 255, 128, 256, 383, 511, 384, 512]

_PROG = None  # cached compiled program


# ---------------------------------------------------------------- host layout


def _key_order(s: int) -> np.ndarray:
    order = []
    for n in range(NT):
        mine = [8 * n + t for t in range(8) if t % 2 == s]
        theirs = [8 * n + t for t in range(8) if t % 2 != s]
        for c in mine + theirs:
            order.extend(range(CH * c, CH * c + CH))
    return np.array(order)


def _masks(s: int) -> np.ndarray:
    ko = _key_order(s)
    qo = np.array([CH * c + i for c in range(s, 64, 2) for i in range(CH)])
    m = np.zeros((8, 128, 512), dtype=np.float32)
    for r in range(8):
        keys = ko[128 * r : 128 * (r + 1)]
        qs = qo[0:512]
        m[r] = np.where(keys[:, None] <= qs[None, :], 0.0, NEG)
    return m


# ---------------------------------------------------------------- bass program


def _build():
    import concourse.mybir as mybir
    import concourse.tile as tile
    from concourse import bacc

    f32 = mybir.dt.float32
    f32r = mybir.dt.float32r
    bf16 = mybir.dt.bfloat16

    nc = bacc.Bacc("TRN2", target_bir_lowering=False, debug=False, num_devices=8)

    xh = nc.dram_tensor("xh", [NT, 128, 8, 512], bf16, kind="ExternalInput").ap()
    wkv = nc.dram_tensor("wkv", [9, 128, 128], bf16, kind="ExternalInput").ap()
    wq = nc.dram_tensor("wq", [9, 128, 64], bf16, kind="ExternalInput").ap()
    wobo = nc.dram_tensor("wobo", [65, 1024], f32r, kind="ExternalInput").ap()
    masks = nc.dram_tensor("masks", [8, 128, 512], bf16, kind="ExternalInput").ap()
    identb = nc.dram_tensor("identb", [128, 128], bf16, kind="ExternalInput").ap()
    ident = nc.dram_tensor("ident", [128, 128], f32r, kind="ExternalInput").ap()
    biases = nc.dram_tensor("biases", [128, 2], f32, kind="ExternalInput").ap()
    vones = nc.dram_tensor("vones", [128, 32, 1], f32r, kind="ExternalInput").ap()
    y = nc.dram_tensor("y", [2048, 1024], f32, kind="ExternalOutput").ap()

    Exp = mybir.ActivationFunctionType.Exp
    Identity = mybir.ActivationFunctionType.Identity
    mult = mybir.AluOpType.mult
    add_op = mybir.AluOpType.add
    scale = 1.0 / math.sqrt(D)

    with tile.TileContext(nc) as tc:
        with (
            tc.tile_pool(name="consts", bufs=1) as consts,
            tc.tile_pool(name="xp", bufs=4) as xpool,
            tc.tile_pool(name="ep", bufs=8) as epool,
            tc.tile_pool(name="yp", bufs=4) as ypool,
            tc.tile_pool(name="pkv", bufs=1, space="PSUM") as pkv,
            tc.tile_pool(name="pq", bufs=1, space="PSUM") as pq,
            tc.tile_pool(name="po", bufs=2, space="PSUM") as po,
            tc.tile_pool(name="ps", bufs=2, space="PSUM") as ps,
        ):
            # ---- constants (one DMA each)
            wkv_sb = consts.tile([128, 9 * 128], bf16)
            nc.gpsimd.dma_start(
                wkv_sb[:].rearrange("p (c f) -> p c f", c=9),
                wkv.rearrange("c p f -> p c f"),
            )
            wq_sb = consts.tile([128, 9 * 64], bf16)
            nc.gpsimd.dma_start(
                wq_sb[:].rearrange("p (c f) -> p c f", c=9),
                wq.rearrange("c p f -> p c f"),
            )
            id_sb = consts.tile([128, 128], f32r)
            nc.gpsimd.dma_start(id_sb[:], ident[:])
            idb_sb = consts.tile([128, 128], bf16)
            nc.gpsimd.dma_start(idb_sb[:], identb[:])
            bias_sb = consts.tile([128, 2], f32)  # col 0: [bk|bov], col 1: bq
            nc.gpsimd.dma_start(bias_sb[:], biases[:])

            kvt_sb = consts.tile([128, N], f32r)  # rows 0:64 KT, 64:128 VT
            qt_sb = consts.tile([H, 2048], f32r)
            v65_sb = consts.tile([128, 32 * 65], f32r)
            nc.gpsimd.dma_start(
                v65_sb[:].rearrange("p (t c) -> p t c", c=65)[:, :, 64:65], vones[:]
            )
            mask_sb = consts.tile([128, 8 * 512], bf16)
            nc.gpsimd.dma_start(
                mask_sb[:].rearrange("p (m f) -> p m f", m=8),
                masks.rearrange("m p f -> p m f"),
            )
            wobo_sb = consts.tile([65, 1024], f32r)
            nc.gpsimd.dma_start(wobo_sb[:], wobo[:])
            ot_sb = consts.tile([65, 2048], f32r)
            recip_sb = consts.tile([128, 16], f32)
            scratch_sb = consts.tile([1, 8], f32)

            # prewarm the ACT exp table while DMAs stream
            nc.scalar.activation(
                scratch_sb[:], id_sb[0:1, 0:8].bitcast(f32), Exp, bias=0.0, scale=1.0
            )

            # ---- interleaved emission: stage A ntiles, with slot j's
            # B/C/D emitted right after ntile 2j+1 so each engine's in-order
            # instruction stream matches data-readiness order.
            def emit_a(n):
                xn = xpool.tile([128, 4096], bf16, tag="x")
                xnv = xn[:].rearrange("p (c f) -> p c f", c=8)
                nc.sync.dma_start(xnv[:], xh[n])
                kvp = pkv.tile([128, 512], f32, tag="kv")
                qp = pq.tile([64, 256], f32, tag="q")
                for c in range(8):
                    nc.tensor.matmul(
                        kvp[:],
                        wkv_sb[:, 128 * c : 128 * (c + 1)],
                        xn[:, 512 * c : 512 * c + 512],
                        start=(c == 0),
                        stop=(c == 7),
                    )
                    nc.tensor.matmul(
                        qp[:],
                        wq_sb[:, 64 * c : 64 * (c + 1)],
                        xn[:, 512 * c : 512 * c + 256],
                        start=(c == 0),
                        stop=(c == 7),
                    )
                nc.vector.tensor_scalar(
                    out=kvt_sb[:, 512 * n : 512 * (n + 1)],
                    in0=kvp[:],
                    scalar1=bias_sb[:, 0:1],
                    scalar2=None,
                    op0=add_op,
                )
                nc.vector.tensor_scalar(
                    out=qt_sb[:, 256 * n : 256 * (n + 1)],
                    in0=qp[:],
                    scalar1=bias_sb[0:64, 1:2],
                    scalar2=None,
                    op0=add_op,
                )
                for t in range(4 * n, 4 * n + 4):
                    vp = po.tile([128, 64], f32r, tag="o")
                    nc.tensor.transpose(
                        vp[:],
                        kvt_sb[64:128, 128 * t : 128 * (t + 1)],
                        id_sb[64:128, 64:128],
                    )
                    nc.vector.tensor_copy(v65_sb[:, 65 * t : 65 * t + 64], vp[:])

            def emit_d(i, wide=False):
                ys = ypool.tile([128, 1024], f32, tag="y")
                if wide:
                    # tail D's: the S rotation is free, use its 2-bank slots
                    # so consecutive i's pipeline instead of serializing on kv
                    yp = ps.tile([128, 1024], f32, tag="s")
                    for d in range(2):
                        nc.tensor.matmul(
                            yp[:, 512 * d : 512 * (d + 1)],
                            ot_sb[:, 128 * i : 128 * (i + 1)],
                            wobo_sb[:, 512 * d : 512 * (d + 1)],
                            start=True,
                            stop=True,
                        )
                    nc.vector.tensor_scalar(
                        out=ys[:],
                        in0=yp[:],
                        scalar1=recip_sb[:, i : i + 1],
                        scalar2=None,
                        op0=mult,
                    )
                else:
                    for d in range(2):
                        yp = pkv.tile([128, 512], f32, tag="kv")
                        nc.tensor.matmul(
                            yp[:],
                            ot_sb[:, 128 * i : 128 * (i + 1)],
                            wobo_sb[:, 512 * d : 512 * (d + 1)],
                            start=True,
                            stop=True,
                        )
                        nc.vector.tensor_scalar(
                            out=ys[:, 512 * d : 512 * (d + 1)],
                            in0=yp[:],
                            scalar1=recip_sb[:, i : i + 1],
                            scalar2=None,
                            op0=mult,
                        )
                nc.gpsimd.dma_start(y[128 * i : 128 * (i + 1), :], ys[:])

            pending_d = []

            bc_state = {}

            def emit_bc(j, t_lo=0, t_hi=None, finish=True):
                nk = 8 * (j + 1)
                if t_hi is None:
                    t_hi = nk
                if j in bc_state:
                    op_ = bc_state[j]
                else:
                    op_ = po.tile([65, 512], f32, tag="o")
                    bc_state[j] = op_
                def emit_c(t0, et, offs):
                    for h in range(2):
                        t = t0 + h
                        off = offs[h]
                        nc.tensor.matmul(
                            op_[:, off:512],
                            v65_sb[:, 65 * t : 65 * (t + 1)],
                            et[:, 512 * h + off : 512 * (h + 1)],
                            start=(t == 0),
                            stop=(t == nk - 1),
                        )

                # software-pipelined: C trails one pair behind B/exp so the
                # PE stream never waits on the exp of the pair it just fed
                prevs = []
                for t0 in range(t_lo, t_hi, 2):
                    if pending_d and t0 % 4 == 0 and t0 > 0:
                        emit_d(pending_d.pop(0))
                    sp = ps.tile([128, 1024], f32, tag="s")
                    offs = []
                    for h in range(2):
                        t = t0 + h
                        rr = t - 8 * j
                        # leading q-columns of a masked tile that are fully
                        # causally dead for BOTH core parities (clamped to 256
                        # so fp32r keeps its N>=256 full rate) - see MASK_OFFS
                        off = 0 if rr < 0 else MASK_OFFS[rr]
                        offs.append(off)
                        nc.tensor.matmul(
                            sp[:, 512 * h + off : 512 * (h + 1)],
                            kvt_sb[0:64, 128 * t : 128 * (t + 1)],
                            qt_sb[:, 512 * j + off : 512 * (j + 1)],
                            start=True,
                            stop=(rr < 0),
                        )
                        if rr >= 0:
                            end = MASK_ENDS[rr]
                            nc.tensor.matmul(
                                sp[:, 512 * h + off : 512 * h + end],
                                idb_sb[:],
                                mask_sb[:, 512 * rr + off : 512 * rr + end],
                                start=False,
                                stop=True,
                            )
                    et = epool.tile([128, 1024], f32r, tag="e")
                    if offs == [0, 0]:
                        nc.scalar.activation(et[:], sp[:], Exp, bias=0.0, scale=scale)
                    elif offs[0] == offs[1]:
                        # one strided activation over both halves' live regions
                        o = offs[0]
                        nc.scalar.activation(
                            et[:].rearrange("p (b f) -> p b f", b=2)[:, :, o:512],
                            sp[:].rearrange("p (b f) -> p b f", b=2)[:, :, o:512],
                            Exp,
                            bias=0.0,
                            scale=scale,
                        )
                    else:
                        for h in range(2):
                            o = 512 * h + offs[h]
                            nc.scalar.activation(
                                et[:, o : 512 * (h + 1)],
                                sp[:, o : 512 * (h + 1)],
                                Exp,
                                bias=0.0,
                                scale=scale,
                            )
                    prevs.append((t0, et, offs))
                    if len(prevs) > 2:
                        emit_c(*prevs.pop(0))
                for p in prevs:
                    emit_c(*p)
                if not finish:
                    return
                nc.vector.tensor_copy(ot_sb[:, 512 * j : 512 * (j + 1)], op_[:])
                rp = pq.tile([128, 4], f32, tag="q")
                for ii in range(4):
                    i = 4 * j + ii
                    nc.tensor.transpose(
                        rp[:, ii : ii + 1],
                        ot_sb[64:65, 128 * i : 128 * (i + 1)].bitcast(f32),
                        id_sb[64:65, 64:65].bitcast(f32),
                    )
                nc.vector.reciprocal(recip_sb[:, 4 * j : 4 * j + 4], rp[:])
                pending_d.extend(range(4 * j, 4 * j + 4))

            for n in range(NT):
                emit_a(n)
                if n % 2 == 1:
                    emit_bc((n - 1) // 2)

            for i in pending_d:
                emit_d(i, wide=True)

    nc.compile()
    return nc


def _get_prog():
    global _PROG
    if _PROG is None:
        _PROG = _build()
    return _PROG


# ---------------------------------------------------------------- entry point


def _xh(xb, korder):
    """[ntile, partition, chunk, 512] bf16 layout of x[b][korder].T."""
    import ml_dtypes

    xt = xb[korder].T  # [1024, 4096]
    return np.ascontiguousarray(
        xt.reshape(8, 128, 8, 512).transpose(2, 1, 0, 3).astype(ml_dtypes.bfloat16)
    )


def kernel(x, Wq, bq, Wk, bk, Wov, bov, Wo, bo, _trace=False):
    from concourse import bass_utils

    x = np.ascontiguousarray(np.asarray(x, dtype=np.float32))
    Wq = np.asarray(Wq, dtype=np.float32)
    bq = np.asarray(bq, dtype=np.float32)
    Wk = np.asarray(Wk, dtype=np.float32)
    bk = np.asarray(bk, dtype=np.float32)
    Wov = np.asarray(Wov, dtype=np.float32)
    bov = np.asarray(bov, dtype=np.float32)
    Wo = np.asarray(Wo, dtype=np.float32)
    bo = np.asarray(bo, dtype=np.float32)

    nc = _get_prog()

    wkv_arr = np.zeros((9, 128, 128), dtype=np.float32)
    wkv_t = np.concatenate([Wk, Wov], axis=0).T  # [1024, 128]
    for c in range(8):
        wkv_arr[c] = wkv_t[128 * c : 128 * (c + 1)]
    wkv_arr[8][0] = np.concatenate([bk, bov])

    wq_arr = np.zeros((9, 128, 64), dtype=np.float32)
    wq_t = Wq.T  # [1024, 64]
    for c in range(8):
        wq_arr[c] = wq_t[128 * c : 128 * (c + 1)]
    wq_arr[8][0] = bq

    import ml_dtypes

    wobo_arr = np.concatenate([Wo.T, bo[None, :]], axis=0)  # [65, 1024]
    wkv_arr = wkv_arr.astype(ml_dtypes.bfloat16)
    wq_arr = wq_arr.astype(ml_dtypes.bfloat16)
    biases_arr = np.zeros((128, 2), dtype=np.float32)
    biases_arr[:, 0] = np.concatenate([bk, bov])
    biases_arr[0:64, 1] = bq
    ident_arr = np.eye(128, dtype=np.float32)
    masks_s = [_masks(0), _masks(1)]
    korder_s = [_key_order(0), _key_order(1)]

    in_maps = []
    for core in range(8):
        b, s = divmod(core, 2)
        in_maps.append(
            {
                "xh": _xh(x[b], korder_s[s]),
                "wkv": wkv_arr,
                "wq": wq_arr,
                "wobo": wobo_arr,
                "masks": masks_s[s].astype(ml_dtypes.bfloat16),
                "identb": ident_arr.astype(ml_dtypes.bfloat16),
                "ident": ident_arr,
                "biases": biases_arr,
                "vones": np.ones((128, 32, 1), dtype=np.float32),
            }
        )

    res = bass_utils.run_bass_kernel_spmd(
        nc, in_maps, core_ids=list(range(8)), trace=_trace
    )

    y = np.empty((B, N, D), dtype=np.float32)
    for core in range(8):
        b, s = divmod(core, 2)
        y[b].reshape(64, CH, D)[s::2] = res.results[core]["y"].reshape(32, CH, D)
    return y



# revision 4
# speedup vs baseline: 1.1043x; 1.1043x over previous
"""Trainium2 Bass kernel for nn_AttentionHead (B=4, n_ctx=4096, d_model=1024,
d_hidden=64, causal, scale=1/sqrt(d_model)).

Sharding: 8 cores = 4 batches x 2 balanced causal shards. Core (b, s) handles
the 2048 query rows in 64-row chunks with chunk%2 == s. Keys/x-columns are
permuted per core (my-parity chunks first within each 512-key ntile) so that
every core runs the IDENTICAL SPMD program:

  - slot j (0..3) = 512 queries = my chunks of ntiles 2j, 2j+1
  - slot j attends k-tiles t = 0..8(j+1)-1 (128 permuted keys each)
  - k-tiles t < 8j are fully open; t = 8j + r (r in 0..7) get an additive
    causal mask that depends only on (r, s) -> 8 mask tiles per core, sent
    as data.

Per-core pipeline:
  A: KT/VT = [Wk;Wov] @ xT (bf16, weights stationary, PSUM-accumulated over
     8 d_model chunks); bias-add -> vt_sb bf16; K rows recast to fp8 (Pool)
     into k8_sb; Q likewise from each ntile's first 256 columns -> q8_sb fp8
     (with a zeroed second half for the DoubleRow trick). V transposed to
     natural [k,64] bf16 layout via PE transpose, with an appended ones
     column so attn@[V,1] also yields the softmax denominator.
  B: S^T[k,q] = K8_tile^T @ Q8_slot in ONE fp8 DoubleRow matmul per k-tile
     (second contraction tile = next K tile x zeros => 0.5 cycles/row, 2x
     over bf16); additive -1e10 mask via bf16 identity matmul for diagonal
     tiles; exp((S+M)/32) on ACT over the pair -> SBUF bf16 (no row-max
     subtraction needed: |scores/32| <~ 1.5).
  C: O65 += V65_tile^T @ E bf16 (PSUM accumulate over k-tiles); row 64 =
     softmax denominator.
  D: y_tile = (O65_slice^T @ [Wo^T; bo]) * (1/den) in f32r -- den row makes
     the matmul add den*bo, so the per-partition reciprocal multiply yields
     O@Wo^T/den + bo exactly; y stored bf16 (host upcasts to f32).

Scheduling: A-stage chunk matmuls for the NEXT two ntiles and pending D
stages are interleaved between B/C pairs inside each slot so the in-order
PE stream never starves while ACT catches up on exp (ACT throughput per
pair ~1.04us vs PE ~0.7us). C trails two pairs behind B/exp. V transposes
are emitted at slot boundaries (po pool shared with the O65 accumulator).
Tail D normalizations run on ACT (idle by then) instead of DVE.
"""

import math

import numpy as np

D = 1024
H = 64
N = 4096
B = 4
CH = 64  # query chunk size (rows)
NT = 8  # ntiles of 512 keys
NEG = -1e10
# per masked-tile r: length of the fully-dead leading q-column prefix (min
# over both core parities). B widens each pair's two windows to the pair
# min so one merged exp per pair reads only matmul-covered PSUM.
TRUE_OFFS = [0, 128, 0, 128, 256, 384, 256, 384]
# per masked-tile r: end of the nonzero mask band (max over parities); the
# mask-add matmul only needs to cover [TRUE_OFFS[r], MASK_ENDS[r])
MASK_ENDS = [127, 255, 128, 256, 383, 511, 384, 512]

_PROG = None  # cached compiled program


# ---------------------------------------------------------------- host layout


def _key_order(s: int) -> np.ndarray:
    order = []
    for n in range(NT):
        mine = [8 * n + t for t in range(8) if t % 2 == s]
        theirs = [8 * n + t for t in range(8) if t % 2 != s]
        for c in mine + theirs:
            order.extend(range(CH * c, CH * c + CH))
    return np.array(order)


def _masks(s: int) -> np.ndarray:
    ko = _key_order(s)
    qo = np.array([CH * c + i for c in range(s, 64, 2) for i in range(CH)])
    m = np.zeros((8, 128, 512), dtype=np.float32)
    for r in range(8):
        keys = ko[128 * r : 128 * (r + 1)]
        qs = qo[0:512]
        m[r] = np.where(keys[:, None] <= qs[None, :], 0.0, NEG)
    return m


# ---------------------------------------------------------------- bass program


def _build():
    import concourse.mybir as mybir
    import concourse.tile as tile
    from concourse import bacc

    f32 = mybir.dt.float32
    f32r = mybir.dt.float32r
    bf16 = mybir.dt.bfloat16
    fp8 = mybir.dt.float8e4

    nc = bacc.Bacc("TRN2", target_bir_lowering=False, debug=False, num_devices=8)

    xh = nc.dram_tensor("xh", [NT, 128, 8, 512], bf16, kind="ExternalInput").ap()
    wkv = nc.dram_tensor("wkv", [9, 128, 128], bf16, kind="ExternalInput").ap()
    wq = nc.dram_tensor("wq", [9, 128, 64], bf16, kind="ExternalInput").ap()
    wobo = nc.dram_tensor("wobo", [65, 1024], f32r, kind="ExternalInput").ap()
    masks = nc.dram_tensor("masks", [8, 128, 512], bf16, kind="ExternalInput").ap()
    identb = nc.dram_tensor("identb", [128, 128], bf16, kind="ExternalInput").ap()
    ident = nc.dram_tensor("ident", [128, 128], f32r, kind="ExternalInput").ap()
    biases = nc.dram_tensor("biases", [128, 2], f32, kind="ExternalInput").ap()
    vones = nc.dram_tensor("vones", [128, 32, 1], bf16, kind="ExternalInput").ap()
    y = nc.dram_tensor("y", [2048, 1024], bf16, kind="ExternalOutput").ap()

    Exp = mybir.ActivationFunctionType.Exp
    Copy = mybir.ActivationFunctionType.Copy
    DR = mybir.MatmulPerfMode.DoubleRow
    mult = mybir.AluOpType.mult
    add_op = mybir.AluOpType.add
    scale = 1.0 / math.sqrt(D)

    with tile.TileContext(nc) as tc:
        with (
            tc.tile_pool(name="consts", bufs=1) as consts,
            tc.tile_pool(name="xp", bufs=4) as xpool,
            tc.tile_pool(name="ep", bufs=8) as epool,
            tc.tile_pool(name="yp", bufs=4) as ypool,
            tc.tile_pool(name="pkv", bufs=1, space="PSUM") as pkv,
            tc.tile_pool(name="pq", bufs=1, space="PSUM") as pq,
            tc.tile_pool(name="po", bufs=2, space="PSUM") as po,
            tc.tile_pool(name="ps", bufs=2, space="PSUM") as ps,
        ):
            # ---- persistent SBUF
            wkv_sb = consts.tile([128, 9 * 128], bf16)
            wq_sb = consts.tile([128, 9 * 64], bf16)
            id_sb = consts.tile([128, 128], f32r)
            idb_sb = consts.tile([128, 128], bf16)
            bias_sb = consts.tile([128, 2], f32)  # col 0: [bk|bov], col 1: bq
            vt_sb = consts.tile([128, N], bf16)  # rows 0:64 KT(bf16), 64:128 VT
            k8_sb = consts.tile([64, N + 128], fp8)  # KT fp8 + finite pad
            q8_sb = consts.tile([64, 2 * 2048], fp8)  # QT fp8 | zeros
            v65_sb = consts.tile([128, 32 * 65], bf16)
            mask_sb = consts.tile([128, 8 * 512], bf16)
            wobo_sb = consts.tile([65, 1024], f32r)
            ot_sb = consts.tile([65, 2048], f32r)
            recip_sb = consts.tile([128, 16], f32)
            scratch_sb = consts.tile([1, 8], f32)

            # DoubleRow zero halves (DVE is idle at start)
            nc.vector.memset(q8_sb[:, 2048:4096], 0.0)
            nc.vector.memset(k8_sb[:, N : N + 128], 0.0)

            # ---- constants (one DMA each, ordered by first use)
            nc.gpsimd.dma_start(
                wkv_sb[:].rearrange("p (c f) -> p c f", c=9),
                wkv.rearrange("c p f -> p c f"),
            )
            nc.gpsimd.dma_start(
                wq_sb[:].rearrange("p (c f) -> p c f", c=9),
                wq.rearrange("c p f -> p c f"),
            )
            nc.gpsimd.dma_start(bias_sb[:], biases[:])
            nc.gpsimd.dma_start(
                mask_sb[:].rearrange("p (m f) -> p m f", m=8),
                masks.rearrange("m p f -> p m f"),
            )
            nc.gpsimd.dma_start(idb_sb[:], identb[:])
            nc.gpsimd.dma_start(
                v65_sb[:].rearrange("p (t c) -> p t c", c=65)[:, :, 64:65], vones[:]
            )
            nc.gpsimd.dma_start(wobo_sb[:], wobo[:])
            nc.gpsimd.dma_start(id_sb[:], ident[:])

            # prewarm the ACT exp table while DMAs stream (reads the zeroed
            # q8 region so no DMA dependency)
            nc.scalar.activation(
                scratch_sb[:], q8_sb[0:1, 2048:2080].bitcast(f32), Exp, bias=0.0,
                scale=1.0,
            )

            k8v = k8_sb[:].rearrange("p (t f) -> p t f", f=128)  # [64, 33, 128]
            q8v = q8_sb[:].rearrange("p (z f) -> p z f", z=2)  # [64, 2, 2048]

            # ---- A stage, chunk-granular so it can interleave into B/C slots
            a_state = {}

            def emit_a_dma(n, split=False):
                xn = xpool.tile([128, 4096], bf16, tag="x")
                xnv = xn[:].rearrange("p (c f) -> p c f", c=8)
                if split:
                    nc.sync.dma_start(xnv[:, 0:4], xh[n, :, 0:4])
                    nc.sync.dma_start(xnv[:, 4:8], xh[n, :, 4:8])
                else:
                    nc.sync.dma_start(xnv[:], xh[n])
                a_state[n] = xn

            def emit_a_chunk(n, c):
                xn = a_state[n]
                if c == 0:
                    a_state[(n, "kv")] = pkv.tile(
                        [128, 512], f32, tag="kv", name="kvp"
                    )
                    a_state[(n, "q")] = pq.tile([64, 256], f32, tag="q", name="qp")
                kvp = a_state[(n, "kv")]
                qp = a_state[(n, "q")]
                nc.tensor.matmul(
                    kvp[:],
                    wkv_sb[:, 128 * c : 128 * (c + 1)],
                    xn[:, 512 * c : 512 * c + 512],
                    start=(c == 0),
                    stop=(c == 7),
                )
                nc.tensor.matmul(
                    qp[:],
                    wq_sb[:, 64 * c : 64 * (c + 1)],
                    xn[:, 512 * c : 512 * c + 256],
                    start=(c == 0),
                    stop=(c == 7),
                )
                if c == 7:
                    nc.vector.tensor_scalar(
                        out=vt_sb[:, 512 * n : 512 * (n + 1)],
                        in0=kvp[:],
                        scalar1=bias_sb[:, 0:1],
                        scalar2=None,
                        op0=add_op,
                    )
                    nc.vector.tensor_scalar(
                        out=q8v[:, 0, 256 * n : 256 * (n + 1)],
                        in0=qp[:],
                        scalar1=bias_sb[0:64, 1:2],
                        scalar2=None,
                        op0=add_op,
                    )
                    # K bf16 -> fp8 recast on Pool (keeps DVE lean)
                    nc.gpsimd.tensor_copy(
                        k8_sb[:, 512 * n : 512 * (n + 1)],
                        vt_sb[0:64, 512 * n : 512 * (n + 1)],
                    )

            def emit_v_tiles(n):
                # V transposes for ntile n (po pool; only at slot boundaries,
                # when no O65 accumulator is live in the pool rotation)
                for t in range(4 * n, 4 * n + 4):
                    vp = po.tile([128, 64], bf16, tag="o")
                    nc.tensor.transpose(
                        vp[:],
                        vt_sb[64:128, 128 * t : 128 * (t + 1)],
                        idb_sb[64:128, 64:128],
                    )
                    nc.vector.tensor_copy(v65_sb[:, 65 * t : 65 * t + 64], vp[:])

            # ---- D stage: output projection for one 128-query row block
            def emit_d(i, on_act=False):
                ys = ypool.tile([128, 1024], bf16, tag="y")
                yp = ps.tile([128, 1024], f32, tag="s")
                for d in range(2):
                    nc.tensor.matmul(
                        yp[:, 512 * d : 512 * (d + 1)],
                        ot_sb[:, 128 * i : 128 * (i + 1)],
                        wobo_sb[:, 512 * d : 512 * (d + 1)],
                        start=True,
                        stop=True,
                    )
                if on_act:
                    nc.scalar.activation(
                        ys[:], yp[:], Copy, bias=0.0, scale=recip_sb[:, i : i + 1]
                    )
                else:
                    nc.vector.tensor_scalar(
                        out=ys[:],
                        in0=yp[:],
                        scalar1=recip_sb[:, i : i + 1],
                        scalar2=None,
                        op0=mult,
                    )
                nc.gpsimd.dma_start(y[128 * i : 128 * (i + 1), :], ys[:])

            pending_d = []

            def emit_bc(j):
                nk = 8 * (j + 1)
                op_ = po.tile([65, 512], f32, tag="o")

                # exp-independent PE filler work, emitted right before each C
                # so the in-order PE stream never waits on ACT
                fillers = []
                if j < 3:
                    emit_a_dma(2 * j + 2)
                    emit_a_dma(2 * j + 3)
                    for n in (2 * j + 2, 2 * j + 3):
                        fillers.extend(("a", n, c) for c in range(8))
                nd = min(len(pending_d), {0: 0, 1: 2, 2: 4, 3: 6}[j])
                fillers.extend(("d", pending_d.pop(0), False) for _ in range(nd))

                npairs = nk // 2

                def emit_fillers(p):
                    want = len(fillers) * (p + 1) // npairs
                    done = a_state.get(("done", j), 0)
                    while done < want:
                        f = fillers[done]
                        if f[0] == "a":
                            emit_a_chunk(f[1], f[2])
                        else:
                            emit_d(f[1])
                        done += 1
                    a_state[("done", j)] = done

                def emit_c(t0, et, offs):
                    for h in range(2):
                        t = t0 + h
                        off = offs[h]
                        nc.tensor.matmul(
                            op_[:, off:512],
                            v65_sb[:, 65 * t : 65 * (t + 1)],
                            et[:, 512 * h + off : 512 * (h + 1)],
                            start=(t == 0),
                            stop=(t == nk - 1),
                        )

                # software-pipelined: C trails two pairs behind B/exp
                prevs = []
                for p in range(npairs):
                    t0 = 2 * p
                    sp = ps.tile([128, 1024], f32, tag="s")
                    offs = []
                    woff = 0  # pair-min B window so one merged exp is safe
                    rr0 = t0 - 8 * j
                    if rr0 >= 0:
                        woff = min(TRUE_OFFS[rr0], TRUE_OFFS[rr0 + 1])
                    for h in range(2):
                        t = t0 + h
                        rr = t - 8 * j
                        offs.append(0 if rr < 0 else TRUE_OFFS[rr])
                        nc.tensor.matmul(
                            sp[:, 512 * h + woff : 512 * (h + 1)],
                            k8v[:, t : t + 2, :],
                            q8v[:, :, 512 * j + woff : 512 * (j + 1)],
                            start=True,
                            stop=(rr < 0),
                            perf_mode=DR,
                        )
                        if rr >= 0:
                            off, end = TRUE_OFFS[rr], MASK_ENDS[rr]
                            nc.tensor.matmul(
                                sp[:, 512 * h + off : 512 * h + end],
                                idb_sb[:],
                                mask_sb[:, 512 * rr + off : 512 * rr + end],
                                start=False,
                                stop=True,
                            )
                    et = epool.tile([128, 1024], bf16, tag="e")
                    if woff == 0:
                        nc.scalar.activation(et[:], sp[:], Exp, bias=0.0, scale=scale)
                    else:
                        nc.scalar.activation(
                            et[:].rearrange("p (b f) -> p b f", b=2)[:, :, woff:512],
                            sp[:].rearrange("p (b f) -> p b f", b=2)[:, :, woff:512],
                            Exp,
                            bias=0.0,
                            scale=scale,
                        )
                    prevs.append((t0, et, offs))
                    emit_fillers(p)
                    if len(prevs) > 2:
                        emit_c(*prevs.pop(0))
                for pr in prevs:
                    emit_c(*pr)
                nc.vector.tensor_copy(ot_sb[:, 512 * j : 512 * (j + 1)], op_[:])
                rp = pq.tile([128, 4], f32, tag="q")
                for ii in range(4):
                    i = 4 * j + ii
                    nc.tensor.transpose(
                        rp[:, ii : ii + 1],
                        ot_sb[64:65, 128 * i : 128 * (i + 1)].bitcast(f32),
                        id_sb[64:65, 64:65].bitcast(f32),
                    )
                nc.vector.reciprocal(recip_sb[:, 4 * j : 4 * j + 4], rp[:])
                pending_d.extend(range(4 * j, 4 * j + 4))

            # ---- emission
            emit_a_dma(0, split=True)
            emit_a_dma(1)
            for c in range(8):
                emit_a_chunk(0, c)
            for c in range(8):
                emit_a_chunk(1, c)
            emit_v_tiles(0)
            emit_v_tiles(1)
            for j in range(4):
                emit_bc(j)
                if j < 3:
                    emit_v_tiles(2 * j + 2)
                    emit_v_tiles(2 * j + 3)

            for i in pending_d:
                emit_d(i, on_act=True)

    nc.compile()
    return nc


def _get_prog():
    global _PROG
    if _PROG is None:
        _PROG = _build()
    return _PROG


# ---------------------------------------------------------------- entry point


def _xh(xb, korder):
    """[ntile, partition, chunk, 512] bf16 layout of x[b][korder].T."""
    import ml_dtypes

    xt = xb[korder].T  # [1024, 4096]
    return np.ascontiguousarray(
        xt.reshape(8, 128, 8, 512).transpose(2, 1, 0, 3).astype(ml_dtypes.bfloat16)
    )


def kernel(x, Wq, bq, Wk, bk, Wov, bov, Wo, bo, _trace=False):
    from concourse import bass_utils

    x = np.ascontiguousarray(np.asarray(x, dtype=np.float32))
    Wq = np.asarray(Wq, dtype=np.float32)
    bq = np.asarray(bq, dtype=np.float32)
    Wk = np.asarray(Wk, dtype=np.float32)
    bk = np.asarray(bk, dtype=np.float32)
    Wov = np.asarray(Wov, dtype=np.float32)
    bov = np.asarray(bov, dtype=np.float32)
    Wo = np.asarray(Wo, dtype=np.float32)
    bo = np.asarray(bo, dtype=np.float32)

    nc = _get_prog()

    wkv_arr = np.zeros((9, 128, 128), dtype=np.float32)
    wkv_t = np.concatenate([Wk, Wov], axis=0).T  # [1024, 128]
    for c in range(8):
        wkv_arr[c] = wkv_t[128 * c : 128 * (c + 1)]
    wkv_arr[8][0] = np.concatenate([bk, bov])

    wq_arr = np.zeros((9, 128, 64), dtype=np.float32)
    wq_t = Wq.T  # [1024, 64]
    for c in range(8):
        wq_arr[c] = wq_t[128 * c : 128 * (c + 1)]
    wq_arr[8][0] = bq

    import ml_dtypes

    wobo_arr = np.concatenate([Wo.T, bo[None, :]], axis=0)  # [65, 1024]
    wkv_arr = wkv_arr.astype(ml_dtypes.bfloat16)
    wq_arr = wq_arr.astype(ml_dtypes.bfloat16)
    biases_arr = np.zeros((128, 2), dtype=np.float32)
    biases_arr[:, 0] = np.concatenate([bk, bov])
    biases_arr[0:64, 1] = bq
    ident_arr = np.eye(128, dtype=np.float32)
    masks_s = [_masks(0), _masks(1)]
    korder_s = [_key_order(0), _key_order(1)]

    in_maps = []
    for core in range(8):
        b, s = divmod(core, 2)
        in_maps.append(
            {
                "xh": _xh(x[b], korder_s[s]),
                "wkv": wkv_arr,
                "wq": wq_arr,
                "wobo": wobo_arr,
                "masks": masks_s[s].astype(ml_dtypes.bfloat16),
                "identb": ident_arr.astype(ml_dtypes.bfloat16),
                "ident": ident_arr,
                "biases": biases_arr,
                "vones": np.ones((128, 32, 1), dtype=ml_dtypes.bfloat16),
            }
        )

    res = bass_utils.run_bass_kernel_spmd(
        nc, in_maps, core_ids=list(range(8)), trace=_trace
    )

    y = np.empty((B, N, D), dtype=np.float32)
    for core in range(8):
        b, s = divmod(core, 2)
        yc = np.asarray(res.results[core]["y"]).astype(np.float32)
        y[b].reshape(64, CH, D)[s::2] = yc.reshape(32, CH, D)
    return y
